# revision 31
# baseline (speedup 1.0000x reference)
"""Bass/Tile TRN2 kernel for nn_MessageAggregation.

Computes: s = sum_n e2[n]; out = leaky_relu((e1+s) @ W1.T + (e1*s) @ W2.T)

Sharding: data-parallel over batch B=8192 across 8 NeuronCores (1024 rows
per core); W1/W2 replicated. Per-core layout: SBUF [128 partitions, 1024
free]; partition p holds batch rows 8p..8p+7 (4 KB contiguous per
partition per DMA descriptor). The kernel is DMA-bound (~32 MB of
all_embeddings2 per core at ~400 GB/s; stream floor ~85 us), so the shape
of the head/tail around the stream is what matters.

Fold-free, DVE-only reduction: all 63 streamed slices accumulate on DVE
into a single SBUF f32r accumulator (1.22 us per [128,1024] slice =
77 us busy < 85 us stream). GpSimd is COMPLETELY idle in-stream - with
it idle, DVE's 3-stream SBUF adds run at full speed; both vector engines
streaming SBUF concurrently with the DMA writes slows the whole machine
~20-30% (measured), which is why earlier fold-based versions split the
work. The stream opens with four singleton 512 KB tiles so DVE's first
add starts ~10.5 us (a 2 MB first tile gave DVE an 8.7 us startup lag it
could never amortize, starving the DMA queue through pool WAR waits).

Stream tail: slices 60-62 arrive as singleton tiles; slice 63 as 8
per-chunk 64 KB DMAs. Final adds: DVE chunks 4-7, GpSimd chunks 0-3
(same SBUF accumulator, disjoint columns, after DVE's slice-62 add).

Tail (no folds): f32r transposes on PE read the accumulator directly,
st copies on scalar (-> f32r), x2t muls on DVE; the f32r matmuls
accumulate onto the head e1@W1.T PSUM group (reopened with start=False):
chunks 4-7 as one 512-wide half (ready first), chunks 0-3 quartered so
the first matmul starts after two copies and the final store is only
128 KB. Lrelu on scalar; stores are issued from the otherwise idle sync
engine. A dummy head lrelu keeps the activation table resident (a
mid-tail ACT_TABLE_LOAD costs 1.3 us).

Free position f = j*128 + p maps to batch row 8p + j; the host gather
un-permutes with a reshape/transpose (not timed).
"""

import sys

for _p in ("/opt/trn_rl_repo",):
    if _p not in sys.path:
        sys.path.insert(0, _p)

import numpy as np

import concourse.bacc as bacc
import concourse.mybir as mybir
import concourse.tile as tile
from concourse.masks import make_identity
from concourse.bass_utils import run_bass_kernel_spmd

B, N, D = 8192, 64, 128
M = 8  # cores
BL = B // M  # 1024 rows per core
R = BL // 128  # chunks per core (8)
F = BL  # free width of the [128, F] working layout
H = F // 2
F32 = mybir.dt.float32
F32R = mybir.dt.float32r
NEG_SLOPE = 0.01
LRELU = mybir.ActivationFunctionType.Lrelu

# Tile sizes for slices 0..62 (slice 63 is chunked): 4 singleton lead-in
# tiles, 14 quads, 3 singleton tail tiles. All slices go to DVE.
TILES = [1, 1, 1, 1] + [4] * 14 + [1, 1, 1]
assert sum(TILES) == N - 1

# Slice-63 chunk DMAs land DVE's chunks first.
CHUNK_DMA_ORDER = [4, 5, 6, 7, 0, 1, 2, 3]


def build(load_bufs: int = 7):
    nc = bacc.Bacc(
        "TRN2",
        target_bir_lowering=False,
        debug=False,
        enable_asserts=False,
        num_devices=M,
    )
    e1 = nc.dram_tensor("embedding1", [BL, D], F32, kind="ExternalInput").ap()
    e2 = nc.dram_tensor("all_embeddings2", [N, BL, D], F32, kind="ExternalInput").ap()
    w1 = nc.dram_tensor("W1", [D, D], F32, kind="ExternalInput").ap()
    w2 = nc.dram_tensor("W2", [D, D], F32, kind="ExternalInput").ap()
    out = nc.dram_tensor("out", [D, BL], F32, kind="ExternalOutput").ap()

    e1_r = e1.rearrange("(p r) d -> p (r d)", p=128)  # [128, 1024]
    e2_r = e2.rearrange("n (p r) d -> p n (r d)", p=128)  # [128, 64, 1024]

    with tile.TileContext(nc) as tc:
        with (
            tc.tile_pool(name="const", bufs=1) as cpool,
            tc.tile_pool(name="load", bufs=load_bufs) as lpool,
            tc.tile_pool(name="lead", bufs=1) as leadpool,
            tc.tile_pool(name="late", bufs=1) as latepool,
            tc.tile_pool(name="last", bufs=8) as lastpool,
            tc.tile_pool(name="act", bufs=1) as apool,
            tc.tile_pool(name="ops", bufs=1, space="PSUM") as opool,
            tc.tile_pool(name="trps", bufs=4, space="PSUM") as trpool,
        ):
            ident = cpool.tile([128, 128], F32)
            make_identity(nc, ident[:])
            ident_r = cpool.tile([128, 128], F32R)
            nc.scalar.copy(out=ident_r[:], in_=ident[:])

            w1_sb = cpool.tile([128, 128], F32)
            nc.scalar.dma_start(out=w1_sb[:], in_=w1)
            w2_sb = cpool.tile([128, 128], F32)
            nc.scalar.dma_start(out=w2_sb[:], in_=w2)
            e1_sb = apool.tile([128, F], F32)
            nc.scalar.dma_start(out=e1_sb[:], in_=e1_r)

            w1t_ps = trpool.tile([128, 128], F32, tag="tr")
            nc.tensor.transpose(w1t_ps[:], w1_sb[:], ident[:])
            w1t = cpool.tile([128, 128], F32)
            nc.scalar.copy(out=w1t[:], in_=w1t_ps[:])
            w1t_r = cpool.tile([128, 128], F32R)
            nc.scalar.copy(out=w1t_r[:], in_=w1t_ps[:])
            w2t_ps = trpool.tile([128, 128], F32, tag="tr")
            nc.tensor.transpose(w2t_ps[:], w2_sb[:], ident[:])
            w2t_r = cpool.tile([128, 128], F32R)
            nc.scalar.copy(out=w2t_r[:], in_=w2t_ps[:])

            # Dummy lrelu at the head so its table is resident for the tail.
            warm = cpool.tile([128, 8], F32)
            nc.scalar.activation(warm[:], ident[:, 0:8], LRELU, alpha=NEG_SLOPE)

            # e1^T pre-stage: chunk j of e1 transposed -> e1t[:, j*128:(j+1)*128]
            e1t = apool.tile([128, F], F32)
            for j in range(R):
                sl = slice(j * 128, (j + 1) * 128)
                tp = trpool.tile([128, 128], F32, tag="tr")
                nc.tensor.transpose(tp[:], e1_sb[:, sl], ident[:])
                nc.scalar.copy(out=e1t[:, sl], in_=tp[:])

            # e1 @ W1.T term of out_T, as a CLOSED accumulation group per
            # half (the tail reopens with start=False).
            o_ps0 = opool.tile([128, H], F32)
            o_ps1 = opool.tile([128, H], F32)
            o_ps = [o_ps0, o_ps1]
            for h in range(2):
                hs = slice(h * H, (h + 1) * H)
                nc.tensor.matmul(
                    o_ps[h][:], lhsT=w1t[:], rhs=e1t[:, hs], start=True, stop=True
                )

            # ---- stream: DVE-only reduction into SBUF accumulator ----
            s_dve = apool.tile([128, F], F32R)
            seen = 0
            base = 0
            for ti, gl in enumerate(TILES):
                if gl == 1 and base < 4:
                    t = leadpool.tile([128, F], F32, tag=f"lead{base}")
                elif gl == 1:
                    t = latepool.tile([128, F], F32, tag=f"late{base}")
                else:
                    t = lpool.tile([128, gl * F], F32, tag="load")
                nc.sync.dma_start(
                    out=t[:].rearrange("p (n f) -> p n f", n=gl),
                    in_=e2_r[:, base : base + gl, :],
                )
                for g in range(gl):
                    sl = t[:, g * F : (g + 1) * F]
                    seen += 1
                    if seen == 1:
                        nc.vector.tensor_copy(out=s_dve[:], in_=sl)
                    else:
                        nc.vector.tensor_add(out=s_dve[:], in0=s_dve[:], in1=sl)
                base += gl

            # Slice 63: 8 per-chunk DMAs; DVE adds 4-7, GpSimd 0-3 (SBUF,
            # disjoint columns, after DVE's slice-62 add).
            last_t = {}
            for c in CHUNK_DMA_ORDER:
                tcch = lastpool.tile([128, 128], F32, tag=f"lc{c}")
                nc.sync.dma_start(
                    out=tcch[:], in_=e2_r[:, N - 1, c * 128 : (c + 1) * 128]
                )
                last_t[c] = tcch
            for c in [4, 5, 6, 7]:
                sl = slice(c * 128, (c + 1) * 128)
                nc.vector.tensor_add(
                    out=s_dve[:, sl], in0=s_dve[:, sl], in1=last_t[c][:]
                )
            for c in [0, 1, 2, 3]:
                sl = slice(c * 128, (c + 1) * 128)
                nc.gpsimd.tensor_add(
                    out=s_dve[:, sl], in0=s_dve[:, sl], in1=last_t[c][:]
                )

            # ---- tail (no folds): transposes read s_dve directly ----
            st = apool.tile([128, F], F32R)
            x2t = apool.tile([128, F], F32R)
            out_sb = apool.tile([128, F], F32)

            tps = {}
            for j in [4, 5, 6, 7, 0, 1, 2, 3]:
                sl = slice(j * 128, (j + 1) * 128)
                tp = trpool.tile([128, 128], F32R, tag="tr")
                nc.tensor.transpose(tp[:], s_dve[:, sl], ident_r[:])
                tps[j] = tp

            # x2t muls per chunk (DVE; tp lives in PSUM so DVE only).
            for j in [4, 5, 6, 7, 0, 1, 2, 3]:
                sl = slice(j * 128, (j + 1) * 128)
                nc.vector.tensor_mul(out=x2t[:, sl], in0=e1t[:, sl], in1=tps[j][:])

            # st copies per chunk on scalar (all before the acts).
            for j in [4, 5, 6, 7, 0, 1, 2, 3]:
                sl = slice(j * 128, (j + 1) * 128)
                nc.scalar.copy(out=st[:, sl], in_=tps[j][:])

            # h1 (chunks 4-7, ready first) as one half; h0 quartered.
            hs = slice(H, F)
            nc.tensor.matmul(
                o_ps[1][:], lhsT=w1t_r[:], rhs=st[:, hs], start=False, stop=False
            )
            nc.tensor.matmul(
                o_ps[1][:], lhsT=w2t_r[:], rhs=x2t[:, hs], start=False, stop=True
            )
            nc.scalar.activation(out_sb[:, hs], o_ps[1][:], LRELU, alpha=NEG_SLOPE)
            nc.sync.dma_start(out=out[:, hs], in_=out_sb[:, hs])
            Q = H // 2
            for q in range(2):
                qs = slice(q * Q, (q + 1) * Q)
                ops_q = o_ps[0][:, qs]
                nc.tensor.matmul(
                    ops_q, lhsT=w1t_r[:], rhs=st[:, qs], start=False, stop=False
                )
                nc.tensor.matmul(
                    ops_q,
                    lhsT=w2t_r[:],
                    rhs=x2t[:, qs],
                    start=False,
                    stop=(q == 1),
                    skip_group_check=True,
                )
                nc.scalar.activation(out_sb[:, qs], ops_q, LRELU, alpha=NEG_SLOPE)
                nc.sync.dma_start(out=out[:, qs], in_=out_sb[:, qs])

    nc.compile()
    return nc


_NC = None


def _get_nc():
    global _NC
    if _NC is None:
        _NC = build()
    return _NC


def _make_in_maps(inputs):
    e1 = np.asarray(inputs["embedding1"], dtype=np.float32)
    e2 = np.asarray(inputs["all_embeddings2"], dtype=np.float32)
    w1 = np.asarray(inputs["W1"], dtype=np.float32)
    w2 = np.asarray(inputs["W2"], dtype=np.float32)
    in_maps = []
    for k in range(M):
        sl = slice(k * BL, (k + 1) * BL)
        in_maps.append(
            {
                "embedding1": np.ascontiguousarray(e1[sl]),
                "all_embeddings2": np.ascontiguousarray(e2[:, sl, :]),
                "W1": w1,
                "W2": w2,
            }
        )
    return in_maps


def _unshard(arr):
    # arr: out_T [o=128, f=1024] with f = j*128 + p <-> batch row 8p + j
    return arr.reshape(128, 8, 128).transpose(2, 1, 0).reshape(BL, D)


def _run(inputs, trace=False, **kwargs):
    nc = _get_nc()
    res = run_bass_kernel_spmd(
        nc, _make_in_maps(inputs), core_ids=list(range(M)), trace=trace, **kwargs
    )
    full = np.concatenate(
        [_unshard(res.results[k]["out"]) for k in range(M)], axis=0
    )
    return full, res


def kernel(**inputs):
    full, _ = _run(inputs)
    return full


# revision 33
# speedup vs baseline: 1.1807x; 1.1807x over previous
"""Bass/Tile TRN2 kernel for nn_MessageAggregation.

Computes: s = sum_n e2[n]; out = leaky_relu((e1+s) @ W1.T + (e1*s) @ W2.T)

Sharding: data-parallel over batch B=8192 across 8 NeuronCores (1024 rows
per core); W1/W2 replicated.

Per-core layout: SBUF [128 partitions, 1024 free]; partition p holds batch
rows 8p..8p+7 (4 KB contiguous per partition per DMA descriptor). The
kernel is DMA-bound (~32 MB of all_embeddings2 per core at ~400 GB/s;
stream floor ~85 us), so the shape of the head/tail around the stream is
what matters.

Stream: the n-reduction is split DVE 42 / GpSimd 21 slices (~1.23 vs
~2.5 us per [128,1024] slice). DVE accumulates in PSUM (1 SBUF read
stream), GpSimd in SBUF; when both engines run 3-stream SBUF ops
concurrently with the DMA writes, the whole machine slows ~20-30%
(measured), so this split is load-bearing. GpSimd also takes slices 56
and 59 so DVE's last adds track the final deliveries (which arrive as
three singleton 512 KB tiles for fine-grained sems).

Slice 63 arrives as 8 per-chunk 64 KB DMAs: final adds stagger per chunk
(DVE chunks 4-7 into the PSUM accumulator right after its last stream
add; GpSimd chunks 0-3 into its SBUF accumulator), so each chunk's fold
starts as soon as its columns are final.

Tail: DVE folds s_dve+s_gps as [128,256] pairs (doubling as the
PSUM->SBUF move), interleaved with its chunk-adds so f45 issues as soon
as a4/a5 land (GpSimd's columns 4-7 are final long before); f32r
transposes on PE, st copies on scalar, x2t muls on DVE. The f32r matmuls
accumulate onto the head e1@W1.T PSUM group (reopened with start=False):
chunks 4-7 as one 512-wide half (ready first), chunks 0-3 quartered so
the first matmul starts after two copies and the final store is only
128 KB. Lrelu on scalar; stores are issued from the otherwise idle sync
engine. A dummy head lrelu keeps the activation table resident (a
mid-tail ACT_TABLE_LOAD costs 1.3 us).

Measured (good clock + uncontended HBM): ~107.7-107.9 us vs ~109.2 us
for the fold-on-DVE/quarter-store baseline. Run-to-run the shared HBM
and core clock vary +-10-20%; under contention both land ~118-134 us.
The tail DAG past the last HBM byte is engine-saturated (DVE, scalar,
PE mutually pacing); local reorders shift its front but conserve the
total.
Free position f = j*128 + p maps to batch row 8p + j; the host gather
un-permutes with a reshape/transpose (not timed).
"""

import sys

for _p in ("/opt/trn_rl_repo",):
    if _p not in sys.path:
        sys.path.insert(0, _p)

import numpy as np

import concourse.bacc as bacc
import concourse.mybir as mybir
import concourse.tile as tile
from concourse.masks import make_identity
from concourse.bass_utils import run_bass_kernel_spmd

B, N, D = 8192, 64, 128
M = 8  # cores
BL = B // M  # 1024 rows per core
R = BL // 128  # chunks per core (8)
F = BL  # free width of the [128, F] working layout
H = F // 2
F32 = mybir.dt.float32
F32R = mybir.dt.float32r
NEG_SLOPE = 0.01
LRELU = mybir.ActivationFunctionType.Lrelu

# Stream routing for slices 0..62 (slice 63 is chunked): D -> DVE, G -> GpSimd.
PLAN = (
    [(4, "DDGD"), (4, "DGDG")] * 7
    + [(4, "GDDG"), (1, "D"), (1, "D"), (1, "D")]
)
assert sum(g for g, _ in PLAN) == N - 1

# Slice-63 chunk DMAs land DVE's chunks first.
CHUNK_DMA_ORDER = [4, 5, 6, 7, 0, 1, 2, 3]


def build(load_bufs: int = 8):
    nc = bacc.Bacc(
        "TRN2",
        target_bir_lowering=False,
        debug=False,
        enable_asserts=False,
        num_devices=M,
    )
    e1 = nc.dram_tensor("embedding1", [BL, D], F32, kind="ExternalInput").ap()
    e2 = nc.dram_tensor("all_embeddings2", [N, BL, D], F32, kind="ExternalInput").ap()
    w1 = nc.dram_tensor("W1", [D, D], F32, kind="ExternalInput").ap()
    w2 = nc.dram_tensor("W2", [D, D], F32, kind="ExternalInput").ap()
    out = nc.dram_tensor("out", [D, BL], F32, kind="ExternalOutput").ap()

    e1_r = e1.rearrange("(p r) d -> p (r d)", p=128)  # [128, 1024]
    e2_r = e2.rearrange("n (p r) d -> p n (r d)", p=128)  # [128, 64, 1024]

    with tile.TileContext(nc) as tc:
        with (
            tc.tile_pool(name="const", bufs=1) as cpool,
            tc.tile_pool(name="load", bufs=load_bufs) as lpool,
            tc.tile_pool(name="last", bufs=8) as lastpool,
            tc.tile_pool(name="act", bufs=1) as apool,
            tc.tile_pool(name="sdve", bufs=1, space="PSUM") as sdpool,
            tc.tile_pool(name="ops", bufs=1, space="PSUM") as opool,
            tc.tile_pool(name="trps", bufs=4, space="PSUM") as trpool,
        ):
            ident = cpool.tile([128, 128], F32)
            make_identity(nc, ident[:])
            ident_r = cpool.tile([128, 128], F32R)
            nc.scalar.copy(out=ident_r[:], in_=ident[:])

            w1_sb = cpool.tile([128, 128], F32)
            nc.scalar.dma_start(out=w1_sb[:], in_=w1)
            w2_sb = cpool.tile([128, 128], F32)
            nc.scalar.dma_start(out=w2_sb[:], in_=w2)
            e1_sb = apool.tile([128, F], F32)
            nc.scalar.dma_start(out=e1_sb[:], in_=e1_r)

            # W.T in SBUF: stationary operand of the output matmuls. fp32
            # for the exact e1-term at the head; f32r for the single-pass
            # tail matmuls.
            w1t_ps = trpool.tile([128, 128], F32, tag="tr")
            nc.tensor.transpose(w1t_ps[:], w1_sb[:], ident[:])
            w1t = cpool.tile([128, 128], F32)
            nc.scalar.copy(out=w1t[:], in_=w1t_ps[:])
            w1t_r = cpool.tile([128, 128], F32R)
            nc.scalar.copy(out=w1t_r[:], in_=w1t_ps[:])
            w2t_ps = trpool.tile([128, 128], F32, tag="tr")
            nc.tensor.transpose(w2t_ps[:], w2_sb[:], ident[:])
            w2t_r = cpool.tile([128, 128], F32R)
            nc.scalar.copy(out=w2t_r[:], in_=w2t_ps[:])

            # Dummy lrelu at the head so its activation table is resident
            # before the tail (a mid-tail ACT_TABLE_LOAD costs 1.3us on the
            # critical path once scalar COPY ops intervene).
            warm = cpool.tile([128, 8], F32)
            nc.scalar.activation(warm[:], ident[:, 0:8], LRELU, alpha=NEG_SLOPE)

            # e1^T pre-stage: chunk j of e1 transposed -> e1t[:, j*128:(j+1)*128]
            e1t = apool.tile([128, F], F32)
            for j in range(R):
                sl = slice(j * 128, (j + 1) * 128)
                tp = trpool.tile([128, 128], F32, tag="tr")
                nc.tensor.transpose(tp[:], e1_sb[:, sl], ident[:])
                nc.scalar.copy(out=e1t[:, sl], in_=tp[:])

            # e1 @ W1.T term of out_T, as a CLOSED accumulation group per
            # half (PE idle during the stream; tail reopens with start=False).
            o_ps0 = opool.tile([128, H], F32)
            o_ps1 = opool.tile([128, H], F32)
            o_ps = [o_ps0, o_ps1]
            for h in range(2):
                hs = slice(h * H, (h + 1) * H)
                nc.tensor.matmul(
                    o_ps[h][:], lhsT=w1t[:], rhs=e1t[:, hs], start=True, stop=True
                )

            # ---- stream ----
            s_dve = sdpool.tile([128, F], F32)  # PSUM accumulator (2 banks)
            s_gps = apool.tile([128, F], F32)  # GpSimd SBUF accumulator
            seen = {"D": 0, "G": 0}
            base = 0
            for gl, routing in PLAN:
                t = lpool.tile([128, gl * F], F32, tag="load")
                nc.sync.dma_start(
                    out=t[:].rearrange("p (n f) -> p n f", n=gl),
                    in_=e2_r[:, base : base + gl, :],
                )
                for g in range(gl):
                    eng = routing[g]
                    seen[eng] += 1
                    sl = t[:, g * F : (g + 1) * F]
                    if eng == "D":
                        if seen["D"] == 1:
                            nc.vector.tensor_copy(out=s_dve[:], in_=sl)
                        else:
                            nc.vector.tensor_add(out=s_dve[:], in0=s_dve[:], in1=sl)
                    else:
                        if seen["G"] == 1:
                            nc.gpsimd.tensor_copy(out=s_gps[:], in_=sl)
                        else:
                            nc.gpsimd.tensor_add(out=s_gps[:], in0=s_gps[:], in1=sl)
                base += gl

            # Last slice: 8 per-chunk DMAs, staggered final adds.
            last_t = {}
            for c in CHUNK_DMA_ORDER:
                tcch = lastpool.tile([128, 128], F32, tag=f"lc{c}")
                nc.sync.dma_start(
                    out=tcch[:], in_=e2_r[:, N - 1, c * 128 : (c + 1) * 128]
                )
                last_t[c] = tcch
            for c in [0, 1, 2, 3]:
                sl = slice(c * 128, (c + 1) * 128)
                nc.gpsimd.tensor_add(
                    out=s_gps[:, sl], in0=s_gps[:, sl], in1=last_t[c][:]
                )

            # ---- tail ----
            s_sb = apool.tile([128, F], F32R)
            st = apool.tile([128, F], F32R)
            x2t = apool.tile([128, F], F32R)
            out_sb = apool.tile([128, F], F32)

            tps = {}

            def chunk_add(c):
                sl = slice(c * 128, (c + 1) * 128)
                nc.vector.tensor_add(
                    out=s_dve[:, sl], in0=s_dve[:, sl], in1=last_t[c][:]
                )

            def fold_pair(a):
                sl = slice(a * 128, (a + 2) * 128)
                nc.vector.tensor_add(
                    out=s_sb[:, sl], in0=s_dve[:, sl], in1=s_gps[:, sl]
                )

            def tr_chunk(j):
                sl = slice(j * 128, (j + 1) * 128)
                tp = trpool.tile([128, 128], F32R, tag="tr")
                nc.tensor.transpose(tp[:], s_sb[:, sl], ident_r[:])
                tps[j] = tp

            # DVE chain interleaved: f45 needs only a4/a5 (GpSimd's columns
            # 4-7 are final since its last stream add), so each fold issues
            # as soon as its own chunk-adds are in - the transpose/copy/mul
            # chain starts ~0.8us earlier than adds-then-folds order.
            chunk_add(4)
            chunk_add(5)
            fold_pair(4)
            chunk_add(6)
            chunk_add(7)
            fold_pair(6)
            fold_pair(0)
            fold_pair(2)
            for j in [4, 5, 6, 7, 0, 1, 2, 3]:
                tr_chunk(j)

            # x2t muls per chunk (DVE; tp lives in PSUM so DVE only).
            for j in [4, 5, 6, 7, 0, 1, 2, 3]:
                sl = slice(j * 128, (j + 1) * 128)
                nc.vector.tensor_mul(out=x2t[:, sl], in0=e1t[:, sl], in1=tps[j][:])

            # st copies per chunk on scalar (all before the acts).
            for j in [4, 5, 6, 7, 0, 1, 2, 3]:
                sl = slice(j * 128, (j + 1) * 128)
                nc.scalar.copy(out=st[:, sl], in_=tps[j][:])

            # h1 (chunks 4-7, ready first) as one half; h0 quartered so its
            # first matmul starts after cp0/cp1 instead of waiting cp3, and
            # the final store is only 128KB.
            hs = slice(H, F)
            nc.tensor.matmul(
                o_ps[1][:], lhsT=w1t_r[:], rhs=st[:, hs], start=False, stop=False
            )
            nc.tensor.matmul(
                o_ps[1][:], lhsT=w2t_r[:], rhs=x2t[:, hs], start=False, stop=True
            )
            nc.scalar.activation(out_sb[:, hs], o_ps[1][:], LRELU, alpha=NEG_SLOPE)
            nc.sync.dma_start(out=out[:, hs], in_=out_sb[:, hs])
            Q = H // 2
            for q in range(2):
                qs = slice(q * Q, (q + 1) * Q)
                ops_q = o_ps[0][:, qs]
                nc.tensor.matmul(
                    ops_q, lhsT=w1t_r[:], rhs=st[:, qs], start=False, stop=False
                )
                nc.tensor.matmul(
                    ops_q,
                    lhsT=w2t_r[:],
                    rhs=x2t[:, qs],
                    start=False,
                    stop=(q == 1),
                    skip_group_check=True,
                )
                nc.scalar.activation(out_sb[:, qs], ops_q, LRELU, alpha=NEG_SLOPE)
                nc.sync.dma_start(out=out[:, qs], in_=out_sb[:, qs])

    nc.compile()
    return nc


_NC = None


def _get_nc():
    global _NC
    if _NC is None:
        _NC = build()
    return _NC


def _make_in_maps(inputs):
    e1 = np.asarray(inputs["embedding1"], dtype=np.float32)
    e2 = np.asarray(inputs["all_embeddings2"], dtype=np.float32)
    w1 = np.asarray(inputs["W1"], dtype=np.float32)
    w2 = np.asarray(inputs["W2"], dtype=np.float32)
    in_maps = []
    for k in range(M):
        sl = slice(k * BL, (k + 1) * BL)
        in_maps.append(
            {
                "embedding1": np.ascontiguousarray(e1[sl]),
                "all_embeddings2": np.ascontiguousarray(e2[:, sl, :]),
                "W1": w1,
                "W2": w2,
            }
        )
    return in_maps


def _unshard(arr):
    # arr: out_T [o=128, f=1024] with f = j*128 + p <-> batch row 8p + j
    return arr.reshape(128, 8, 128).transpose(2, 1, 0).reshape(BL, D)


def _run(inputs, trace=False, **kwargs):
    nc = _get_nc()
    res = run_bass_kernel_spmd(
        nc, _make_in_maps(inputs), core_ids=list(range(M)), trace=trace, **kwargs
    )
    full = np.concatenate(
        [_unshard(res.results[k]["out"]) for k in range(M)], axis=0
    )
    return full, res


def kernel(**inputs):
    full, _ = _run(inputs)
    return full
